# revision 1
# baseline (speedup 1.0000x reference)
"""
DeepAttMISL segment-reduce kernel for Trainium2 (Bass/Tile), 8 NeuronCores.

Math (see reference):
  h        = relu(x @ W1.T + b1)                    x:[N,1024] -> h:[N,256]
  seg      = segment_sum(h, cluster_id, 8)          -> [8,256]
  h_clust  = seg / max(counts,1)
  h_path   = relu(h_clust @ Wf.T + bf)
  A        = softmax((tanh(h_path@Wa.T+ba) * sigmoid(h_path@Wb.T+bb)) @ Wc.T + bc)
  H        = A @ h_path                             -> [1,256]

Sharding: rows (instances) N=65536 split across 8 cores (8192 each).
Each core computes h for its shard and accumulates the per-cluster
segment sums DIRECTLY in transposed layout segT[hid, k] (h-tile
stationary, one-hot moving), so no PE transposes are needed before the
head.  The 8KB partial is AllReduced (ncfw); a dummy collective at
t~0 pre-warms ncfw so the real one starts fast.  (A p2p
remote_dma_broadcast exchange was tried and is architecturally better,
but the SWDGE desc-gen instruction faults under this axon/fake-NRT
runtime.)  Every core redundantly computes the tiny attention head;
core 0's output is returned (host reshapes [128,2] -> [1,256]).

Precision: big matmul in bf16 (inputs rounded once on host) with fp32
PSUM accumulation; everything downstream fp32.  sigmoid(y) is computed
as 0.5*(1+tanh(y/2)) (0.5 folded into Wb/bb/Wc on host) so the whole
kernel only needs relu/tanh/exp -- all in one ACT table set, no
mid-kernel table reloads.  bc is dropped (softmax shift-invariant).
"""

import sys

if "/opt/trn_rl_repo" not in sys.path:
    sys.path.insert(0, "/opt/trn_rl_repo")

import numpy as np
import ml_dtypes

import concourse.bass as bass
import concourse.tile as tile
from concourse import bacc, mybir
from concourse import bass_utils

ALU = mybir.AluOpType

N_CORES = 8
N_TOTAL = 65536
N_SHARD = N_TOTAL // N_CORES          # 8192 rows per core
DIN = 1024
DHID = 256
K_CL = 8                               # clusters
KC = DIN // 128                        # 8 contraction chunks
ROWT = N_SHARD // 128                  # 64 row-tiles of 128 rows
SB_SIZES = [256, 512, 1280, 1536, 1536, 1536, 1536]  # x superblocks (ramped)
assert sum(SB_SIZES) == N_SHARD
SEG_DELAY = 3                          # row-tiles between h and its seg MMs

# head-const blob layout (f32 elements per partition)
OFF_WFT = 0                            # Wf.T tiled   [2,256] -> 512
OFF_WAT = 512                          # Wa.T tiled   [2,256] -> 512
OFF_WBT = 1024                         # (Wb/2).T     [2,256] -> 512
OFF_WCR = 1536                         # (Wc/2) bcast [2,128] -> 256
OFF_BFC = 1792                         # bf           [2]
OFF_BAC = 1794                         # ba           [2]
OFF_BBC = 1796                         # bb/2         [2]
OFF_INV = 1798                         # 1/count tiled x2 [16]
NBLOB = 1814

BF16 = mybir.dt.bfloat16
F32 = mybir.dt.float32
AF = mybir.ActivationFunctionType

_CACHE = {}


def _build_nc():
    nc = bacc.Bacc("TRN2", target_bir_lowering=False, debug=False,
                   num_devices=N_CORES)

    xT = nc.dram_tensor("xT", [DIN, N_SHARD], BF16, kind="ExternalInput")
    w1t = nc.dram_tensor("w1t", [DIN, DHID], BF16, kind="ExternalInput")
    moh = nc.dram_tensor("moh", [128, ROWT, K_CL], BF16, kind="ExternalInput")
    b1b = nc.dram_tensor("b1b", [128, DHID], F32, kind="ExternalInput")
    blob = nc.dram_tensor("blob", [128, NBLOB], F32, kind="ExternalInput")

    out = nc.dram_tensor("out", [128, 2], F32, kind="ExternalOutput")

    with tile.TileContext(nc) as tc:
        with tc.tile_pool(name="consts", bufs=1) as consts, \
             tc.tile_pool(name="xblk", bufs=1) as xblk, \
             tc.tile_pool(name="hpool", bufs=6) as hpool, \
             tc.tile_pool(name="hps", bufs=4, space="PSUM") as hps, \
             tc.tile_pool(name="segps", bufs=1, space="PSUM") as segps, \
             tc.tile_pool(name="headps", bufs=2, space="PSUM") as headps, \
             tc.tile_pool(name="small", bufs=1) as small, \
             tc.tile_pool(name="dram", bufs=1, space="DRAM") as dram:

            # ---- dummy collective, triggered as early as possible: ncfw's
            # cold wake latency is large and variable (11..60+us observed);
            # paying it here, overlapped with the main loop, makes the real
            # AllReduce start ~1us after its trigger.  Nothing reads the
            # output, so no core blocks on it.
            warm_z = small.tile([128, 1], F32)
            nc.vector.memset(warm_z[:], 0.0)
            wcc_in = dram.tile([128, 1], F32)
            wcc_out = dram.tile([128, 1], F32)
            nc.sync.dma_start(wcc_in[:], warm_z[:])
            nc.gpsimd.collective_compute(
                "AllReduce", ALU.add,
                replica_groups=[list(range(N_CORES))],
                ins=[wcc_in[:].opt()], outs=[wcc_out[:].opt()])

            # ---- PE warm-up: dummy matmuls sized to bridge the FULL gap
            # until superblock 0 lands (~24us): 64 x 512-wide at mixed
            # cold/warm clock ends right as real data arrives, so HAM stays
            # un-throttled (2.4GHz) into the loop instead of re-ramping.
            # (A 48-MM block ended ~6us early, re-throttled, and measured
            # neutral -- the bridge must reach the data.)
            wz = consts.tile([128, 512], BF16)
            nc.vector.memset(wz[:], 0.0)
            wps = hps.tile([128, 512], F32, tag="hp")
            for _ in range(64):
                nc.tensor.matmul(wps[:], wz[:, 0:128], wz[:],
                                 start=True, stop=True, skip_group_check=True)

            # ---- critical consts: W1.T split across both HWDGE rings,
            # loaded per k-chunk interleaved with sb0's chunks below so the
            # first matmul isn't gated behind a monolithic 512KB transfer
            w1t_sb = consts.tile([128, KC, DHID], BF16)
            w1v = w1t.ap().rearrange("(k p) f -> p k f", p=128)

            # ---- x superblock tiles: the whole 16MiB shard fits in SBUF
            # (128KB/partition), so allocate every block statically and
            # issue ALL the DMAs upfront -- no buffer reuse means no
            # issue-side flow control, and the rings stream back-to-back
            # at full HBM rate instead of being locked to PE consumption.
            xts_blocks = []
            for sb, sbr in enumerate(SB_SIZES):
                xts = xblk.tile([128, KC, sbr], BF16, name=f"xts{sb}")
                xts_blocks.append(xts)

            RING_KS = {"sync": (0, 2, 4, 6), "scalar": (1, 3, 5, 7)}

            def issue_sb(sb, which):
                sbr = SB_SIZES[sb]
                r0 = sum(SB_SIZES[:sb])
                eng = {"sync": nc.sync, "scalar": nc.scalar}[which]
                for k in RING_KS[which]:
                    eng.dma_start(xts_blocks[sb][:, k, :],
                                  xT.ap()[k * 128:(k + 1) * 128,
                                          r0:r0 + sbr])

            # sync ring: its x half all upfront (static buffers, no waits,
            # streams back-to-back at ring rate).  The scalar ring's half is
            # paced inside the loop: the ACT engine also runs the RELUs, and
            # pushing 8MiB of descriptors upfront stalls it on ring space,
            # which backs up PSUM drains and stalls the PE.
            for k in range(KC):
                eng = nc.sync if k % 2 == 0 else nc.scalar
                eng.dma_start(w1t_sb[:, k, :], w1v[:, k, :])
                eng.dma_start(xts_blocks[0][:, k, :],
                              xT.ap()[k * 128:(k + 1) * 128, 0:SB_SIZES[0]])
            b1b_sb = consts.tile([128, DHID], F32)
            nc.sync.dma_start(b1b_sb[:], b1b.ap())
            m_sb = consts.tile([128, ROWT, K_CL], BF16)
            nc.scalar.dma_start(m_sb[:], moh.ap())
            issue_sb(1, "scalar")
            for sb in range(1, len(SB_SIZES)):
                issue_sb(sb, "sync")

            # ---- transposed segment-sum accumulators (live whole loop).
            # One PSUM bank PER group: a matmul's start=True clears the
            # whole bank's has_written bits, so two interleaved accumulation
            # groups in one bank lose the first group's opening tile.
            segT = [segps.tile([128, K_CL], F32, padded_shape=[128, 128],
                               name=f"segT{j}") for j in range(2)]

            def emit_seg(ph, pt):
                for j in range(2):
                    nc.tensor.matmul(
                        segT[j][:],
                        ph[:, j * 128:(j + 1) * 128], m_sb[:, pt, :],
                        start=(pt == 0), stop=(pt == ROWT - 1),
                        skip_group_check=True)

            # ---- main loop (scalar ring's x half paced one sb ahead) ----
            pending = []
            row0 = 0
            for sb, sbr in enumerate(SB_SIZES):
                xts = xts_blocks[sb]
                if 1 <= sb < len(SB_SIZES) - 1:
                    issue_sb(sb + 1, "scalar")
                for tl in range(sbr // 128):
                    t = row0 // 128 + tl
                    hp = hps.tile([128, DHID], F32, tag="hp")
                    for k in range(KC):
                        nc.tensor.matmul(
                            hp[:],
                            xts[:, k, tl * 128:(tl + 1) * 128],
                            w1t_sb[:, k, :],
                            start=(k == 0), stop=(k == KC - 1),
                            skip_group_check=True)
                    nc.vector.tensor_add(hp[:], hp[:], b1b_sb[:])
                    h_sb = hpool.tile([128, DHID], BF16)
                    nc.scalar.activation(h_sb[:], hp[:], AF.Relu)
                    pending.append((h_sb, t))
                    if len(pending) > SEG_DELAY:
                        ph, pt = pending.pop(0)
                        emit_seg(ph, pt)
                row0 += sbr
            while pending:
                ph, pt = pending.pop(0)
                emit_seg(ph, pt)

            # head consts: issued after ALL x DMAs on the sync ring so they
            # never delay the x stream (ring is FIFO); land ~15us before use
            blob_sb = consts.tile([128, NBLOB], F32)
            nc.sync.dma_start(blob_sb[:], blob.ap())

            # ---- AllReduce the 8KB transposed partial across the 8 cores ----
            seg_loc = small.tile([128, 2 * K_CL], F32)
            nc.vector.tensor_copy(seg_loc[:, 0:K_CL], segT[0][:])
            nc.vector.tensor_copy(seg_loc[:, K_CL:2 * K_CL], segT[1][:])
            ar_in = dram.tile([128, 2 * K_CL], F32)
            ar_out = dram.tile([128, 2 * K_CL], F32)
            nc.sync.dma_start(ar_in[:], seg_loc[:])
            nc.gpsimd.collective_compute(
                "AllReduce", ALU.add,
                replica_groups=[list(range(N_CORES))],
                ins=[ar_in[:].opt()], outs=[ar_out[:].opt()])
            tot = small.tile([128, 2 * K_CL], F32)
            nc.sync.dma_start(tot[:], ar_out[:])

            # ---- cluster means (1/count varies along free dim) ----
            hcT = small.tile([128, 2 * K_CL], F32)
            nc.vector.tensor_mul(hcT[:], tot[:],
                                 blob_sb[:, OFF_INV:OFF_INV + 16])

            # ---- attention head, transposed layout [hid(2x128), k] ----
            def head_mm(w_off, rhs, b_off, func, name):
                o = small.tile([128, 2 * K_CL], F32, name=name)
                for j in range(2):
                    ps = headps.tile([128, K_CL], F32, tag="head",
                                     padded_shape=[128, 128])
                    for i in range(2):
                        nc.tensor.matmul(
                            ps[:],
                            blob_sb[:, w_off + i * 256 + j * 128:
                                    w_off + i * 256 + (j + 1) * 128],
                            rhs[:, i * K_CL:(i + 1) * K_CL],
                            start=(i == 0), stop=(i == 1))
                    nc.scalar.activation(o[:, j * K_CL:(j + 1) * K_CL], ps[:],
                                         func,
                                         bias=blob_sb[:, b_off + j:b_off + j + 1])
                return o

            hpT = head_mm(OFF_WFT, hcT, OFF_BFC, AF.Relu, "hpT")
            aT = head_mm(OFF_WAT, hpT, OFF_BAC, AF.Tanh, "aT")
            tT = head_mm(OFF_WBT, hpT, OFF_BBC, AF.Tanh, "tT")
            # a*g = a*0.5*(1+tanh) ; the 0.5 lives in Wc/2
            ag = small.tile([128, 2 * K_CL], F32)
            nc.vector.tensor_mul(ag[:], aT[:], tT[:])
            nc.vector.tensor_add(ag[:], ag[:], aT[:])

            # logits replicated on all 128 partitions (bc dropped: softmax
            # is shift-invariant)
            a_ps = headps.tile([128, K_CL], F32, tag="head",
                               padded_shape=[128, 128])
            for j in range(2):
                nc.tensor.matmul(
                    a_ps[:],
                    blob_sb[:, OFF_WCR + j * 128:OFF_WCR + (j + 1) * 128],
                    ag[:, j * K_CL:(j + 1) * K_CL],
                    start=(j == 0), stop=(j == 1))

            # softmax over 8 clusters (bounded logits; skip max-shift);
            # exp+rowsum fused via accum_out
            ea = small.tile([128, K_CL], F32)
            ssum = small.tile([128, 1], F32)
            nc.scalar.activation(ea[:], a_ps[:], AF.Exp, accum_out=ssum[:])
            rs = small.tile([128, 1], F32)
            nc.vector.reciprocal(rs[:], ssum[:])

            # H[hid] = (sum_k ea[k]*h_path.T[hid,k]) / sum_k ea[k].
            # Normalizing AFTER the reduce is algebraically identical and
            # takes the reciprocal off the critical chain (it overlaps the
            # muls) while dropping one DVE op.
            h_un = small.tile([128, 2], F32)
            for j in range(2):
                tmp = small.tile([128, K_CL], F32, name=f"wtmp{j}")
                nc.vector.tensor_mul(tmp[:], hpT[:, j * K_CL:(j + 1) * K_CL],
                                     ea[:])
                nc.vector.reduce_sum(h_un[:, j:j + 1], tmp[:],
                                     axis=mybir.AxisListType.X)
            h_out = small.tile([128, 2], F32)
            nc.vector.tensor_scalar_mul(h_out[:], h_un[:], rs[:, 0:1])
            nc.sync.dma_start(out.ap()[:, :], h_out[:])

    nc.compile()
    return nc


def _prep_inputs(x_path, cluster_id, W1, b1, Wf, bf, Wa, ba, Wb, bb, Wc, bc):
    """Host-side sharding / marshalling. Returns in_maps for the 8 cores."""
    x = np.asarray(x_path, dtype=np.float32).reshape(N_TOTAL, DIN)
    cid = np.asarray(cluster_id).astype(np.int64).reshape(N_TOTAL)

    xb = x.astype(ml_dtypes.bfloat16)

    # one-hot cluster matrix, pre-tiled to [128, ROWT, K] per core
    oh = (cid[:, None] == np.arange(K_CL)[None, :]).astype(ml_dtypes.bfloat16)

    counts = np.bincount(cid, minlength=K_CL).astype(np.float32)
    invc = (1.0 / np.maximum(counts, 1.0)).astype(np.float32)

    W1 = np.asarray(W1, np.float32); b1 = np.asarray(b1, np.float32)
    Wf = np.asarray(Wf, np.float32); bf = np.asarray(bf, np.float32)
    Wa = np.asarray(Wa, np.float32); ba = np.asarray(ba, np.float32)
    Wb = np.asarray(Wb, np.float32); bb = np.asarray(bb, np.float32)
    Wc = np.asarray(Wc, np.float32)

    def tiled_T(M):  # [256,256] -> [128, 2, 256]; [p,i,f] = M.T[i*128+p, f]
        return np.ascontiguousarray(M.T.reshape(2, 128, DHID).transpose(1, 0, 2))

    blob = np.zeros((128, NBLOB), np.float32)
    blob[:, OFF_WFT:OFF_WFT + 512] = tiled_T(Wf).reshape(128, 512)
    blob[:, OFF_WAT:OFF_WAT + 512] = tiled_T(Wa).reshape(128, 512)
    blob[:, OFF_WBT:OFF_WBT + 512] = tiled_T(Wb * 0.5).reshape(128, 512)
    # (Wc/2) broadcast: [q, j*128+c] = Wc[0, j*128+q]/2 for all c
    wcr = np.broadcast_to((Wc.ravel() * 0.5).reshape(2, 128, 1),
                          (2, 128, 128)).transpose(1, 0, 2)
    blob[:, OFF_WCR:OFF_WCR + 256] = wcr.reshape(128, 256)
    blob[:, OFF_BFC:OFF_BFC + 2] = bf.reshape(2, 128).T
    blob[:, OFF_BAC:OFF_BAC + 2] = ba.reshape(2, 128).T
    blob[:, OFF_BBC:OFF_BBC + 2] = (bb * 0.5).reshape(2, 128).T
    blob[:, OFF_INV:OFF_INV + 16] = np.tile(invc, 2)[None, :]

    const_map = {
        "w1t": np.ascontiguousarray(W1.T).astype(ml_dtypes.bfloat16),
        "b1b": np.ascontiguousarray(np.broadcast_to(b1, (128, DHID))),
        "blob": blob,
    }

    in_maps = []
    for c in range(N_CORES):
        lo, hi = c * N_SHARD, (c + 1) * N_SHARD
        xT_c = np.ascontiguousarray(xb[lo:hi].T)            # [1024, 8192] bf16
        moh_c = np.ascontiguousarray(
            oh[lo:hi].reshape(ROWT, 128, K_CL).transpose(1, 0, 2))
        in_maps.append({"xT": xT_c, "moh": moh_c, **const_map})
    return in_maps


def kernel(**inputs):
    if "nc" not in _CACHE:
        _CACHE["nc"] = _build_nc()
    nc = _CACHE["nc"]
    in_maps = _prep_inputs(**inputs)
    res = bass_utils.run_bass_kernel_spmd(
        nc, in_maps, core_ids=list(range(N_CORES)))
    o = res.results[0]["out"]                               # [128, 2]
    return np.ascontiguousarray(o.T.reshape(1, DHID)).astype(np.float32)



# revision 3
# speedup vs baseline: 1.8991x; 1.8991x over previous
"""
DeepAttMISL segment-reduce kernel for Trainium2 (Bass/Tile), 8 NeuronCores.

Math (see reference):
  h        = relu(x @ W1.T + b1)                    x:[N,1024] -> h:[N,256]
  seg      = segment_sum(h, cluster_id, 8)          -> [8,256]
  h_clust  = seg / max(counts,1)
  h_path   = relu(h_clust @ Wf.T + bf)
  A        = softmax((tanh(h_path@Wa.T+ba) * sigmoid(h_path@Wb.T+bb)) @ Wc.T)
  H        = A @ h_path                             -> [1,256]

Sharding: BY CLUSTER, not by rows.  Core k receives ALL rows of cluster k
(host sorts rows by cluster_id), zero-padded to a fixed NPAD rows.  Each
core therefore owns its cluster's full segment sum locally and NO cross-core
collective is needed (the ncfw AllReduce costs 25-35us per op in this
runtime, plus a ~56us wake, and dominated the previous version's critical
path).  Each core runs the tiny gated-attention head for its own cluster and
outputs (logit_k, h_path_k); the host does the final 8-way softmax +
weighted sum as the gather/unshard step.

Main matmul is computed TRANSPOSED (W1 stationary, x moving, h.T in PSUM
[hid_half, rows]) so the segment sum falls out of ACT's accum_out: one
activation op per PSUM tile does bias + relu + sum-over-rows.  No segment
matmuls, no one-hot matrix.  Zero-pad rows contribute exactly relu(b1)
each; the host bakes -n_pad*relu(b1)/count into a per-core correction.

x is streamed as NBLK contiguous 1MiB DMAs (8KiB per partition per block)
on the sync ring - near line rate.  bf16 everywhere in the big matmul
(fp8 fails the 2e-2 gate: W1's quantization error is shared across
instances so it does not average out); fp32 head.  sigmoid(y) =
0.5*(1+tanh(y/2)) with the 0.5 folded into Wc so one ACT table set
(relu/tanh/exp) serves the whole kernel.
"""

import sys

if "/opt/trn_rl_repo" not in sys.path:
    sys.path.insert(0, "/opt/trn_rl_repo")

import numpy as np
import ml_dtypes

import concourse.bass as bass
import concourse.tile as tile
from concourse import bacc, mybir
from concourse import bass_utils

ALU = mybir.AluOpType

N_CORES = 8
N_TOTAL = 65536
DIN = 1024
DHID = 256
K_CL = 8
KC = DIN // 128                        # 8 contraction chunks of 128
BLK = 512                              # rows per block (PSUM bank = 512 fp32)
WARMUP_MMS = 52                        # PE bridge until first x block lands

# blob layout (fp32 elements per partition)
OFF_WFT = 0                            # Wf.T tiled   [2,256] -> 512
OFF_WAT = 512                          # Wa.T tiled   [2,256] -> 512
OFF_WBT = 1024                         # (Wb/2).T     [2,256] -> 512
OFF_WCR = 1536                         # (Wc/2) bcast [2,128] -> 256
OFF_BFC = 1792                         # bf tiled     [2]
OFF_BAC = 1794                         # ba tiled     [2]
OFF_BBC = 1796                         # bb/2 tiled   [2]
OFF_B1C = 1798                         # b1 tiled     [2]
OFF_CORR = 1800                        # -invc*n_pad*relu(b1) tiled [2]
OFF_INVC = 1802                        # 1/max(count,1) scalar [1]
NBLOB = 1803

BF16 = mybir.dt.bfloat16
F32 = mybir.dt.float32
AF = mybir.ActivationFunctionType

_CACHE = {}


def _build_nc(nblk):
    npad = nblk * BLK
    nc = bacc.Bacc("TRN2", target_bir_lowering=False, debug=False,
                   num_devices=N_CORES)

    xb = nc.dram_tensor("xb", [128, nblk * KC * BLK], BF16,
                        kind="ExternalInput")
    w1t = nc.dram_tensor("w1t", [128, KC * DHID], BF16, kind="ExternalInput")
    blob = nc.dram_tensor("blob", [128, NBLOB], F32, kind="ExternalInput")
    out = nc.dram_tensor("out", [128, 4], F32, kind="ExternalOutput")

    with tile.TileContext(nc) as tc:
        with tc.tile_pool(name="consts", bufs=1) as consts, \
             tc.tile_pool(name="xblk", bufs=1) as xblk, \
             tc.tile_pool(name="hps", bufs=4, space="PSUM") as hps, \
             tc.tile_pool(name="headps", bufs=2, space="PSUM") as headps, \
             tc.tile_pool(name="small", bufs=1) as small:

            # ---- PE warm-up bridge: keep HAM busy (and un-throttled by the
            # time real data arrives) from t~0 until block 0 lands (~13us).
            wz = consts.tile([128, BLK], BF16)
            nc.vector.memset(wz[:], 0.0)
            wps = hps.tile([128, BLK], F32, tag="main")
            for _ in range(WARMUP_MMS):
                nc.tensor.matmul(wps[:], wz[:, 0:128], wz[:],
                                 start=True, stop=True, skip_group_check=True)

            # ---- DMAs: consts on the scalar ring (tiny bias slice first so
            # the first ACT isn't gated on the 1MiB head-weight blob, then
            # W1, then the head weights), all x blocks on the sync ring as
            # back-to-back 1MiB transfers (8KiB/partition lines).
            blob_sb = consts.tile([128, NBLOB], F32)
            nc.scalar.dma_start(blob_sb[:, OFF_BFC:], blob.ap()[:, OFF_BFC:])
            w1t_sb = consts.tile([128, KC, DHID], BF16)
            nc.scalar.dma_start(w1t_sb[:], w1t.ap().rearrange(
                "p (k f) -> p k f", k=KC))
            nc.scalar.dma_start(blob_sb[:, :OFF_BFC], blob.ap()[:, :OFF_BFC])

            xts = xblk.tile([128, nblk, KC, BLK], BF16)
            xv = xb.ap().rearrange("p (b k r) -> p b k r", b=nblk, k=KC)
            for b in range(nblk):
                nc.sync.dma_start(xts[:, b], xv[:, b])

            # ---- main loop: per block, per hid-half: 8 accumulated MMs
            # (W1 chunk stationary, x moving, N=512), then one ACT op doing
            # bias + relu + accum_out (the segment sum over this block).
            segparts = small.tile([128, 2, nblk], F32)
            hsc = [small.tile([128, BLK], BF16, name=f"hsc{i}")
                   for i in range(3)]
            for b in range(nblk):
                for j in range(2):
                    ps = hps.tile([128, BLK], F32, tag="main")
                    for c in range(KC):
                        nc.tensor.matmul(
                            ps[:],
                            w1t_sb[:, c, j * 128:(j + 1) * 128],
                            xts[:, b, c, :],
                            start=(c == 0), stop=(c == KC - 1),
                            skip_group_check=True)
                    nc.scalar.activation(
                        hsc[(2 * b + j) % 3][:], ps[:], AF.Relu,
                        bias=blob_sb[:, OFF_B1C + j:OFF_B1C + j + 1],
                        accum_out=segparts[:, j, b:b + 1])

            # ---- local segment sum -> cluster mean (with padding fix) ----
            seg2 = small.tile([128, 2], F32)
            for j in range(2):
                nc.vector.reduce_sum(seg2[:, j:j + 1], segparts[:, j, :],
                                     axis=mybir.AxisListType.X)
            hc = small.tile([128, 2], F32)
            nc.vector.tensor_scalar_mul(hc[:], seg2[:],
                                        blob_sb[:, OFF_INVC:OFF_INVC + 1])
            nc.vector.tensor_add(hc[:], hc[:],
                                 blob_sb[:, OFF_CORR:OFF_CORR + 2])

            # ---- gated-attention head for this core's cluster (fp32) ----
            def head_mm(w_off, rhs, b_off, func, name):
                o = small.tile([128, 2], F32, name=name)
                for j in range(2):
                    ps = headps.tile([128, 1], F32, tag="head",
                                     padded_shape=[128, BLK])
                    for i in range(2):
                        nc.tensor.matmul(
                            ps[:],
                            blob_sb[:, w_off + i * 256 + j * 128:
                                    w_off + i * 256 + (j + 1) * 128],
                            rhs[:, i:i + 1],
                            start=(i == 0), stop=(i == 1))
                    nc.scalar.activation(o[:, j:j + 1], ps[:], func,
                                         bias=blob_sb[:, b_off + j:
                                                      b_off + j + 1])
                return o

            hpT = head_mm(OFF_WFT, hc, OFF_BFC, AF.Relu, "hpT")
            aT = head_mm(OFF_WAT, hpT, OFF_BAC, AF.Tanh, "aT")
            tT = head_mm(OFF_WBT, hpT, OFF_BBC, AF.Tanh, "tT")
            # a*g = 0.5*a*(1+tanh(y/2)); the 0.5 lives in Wc/2
            ag = small.tile([128, 2], F32)
            nc.vector.tensor_mul(ag[:], aT[:], tT[:])
            nc.vector.tensor_add(ag[:], ag[:], aT[:])

            # logit (replicated across partitions via broadcast Wc/2)
            lps = headps.tile([128, 1], F32, tag="head",
                              padded_shape=[128, BLK])
            for j in range(2):
                nc.tensor.matmul(
                    lps[:],
                    blob_sb[:, OFF_WCR + j * 128:OFF_WCR + (j + 1) * 128],
                    ag[:, j:j + 1],
                    start=(j == 0), stop=(j == 1))

            outsb = small.tile([128, 4], F32)
            nc.vector.tensor_copy(outsb[:, 0:2], hpT[:])
            nc.vector.tensor_copy(outsb[:, 2:3], lps[:])
            nc.vector.memset(outsb[:, 3:4], 0.0)
            nc.sync.dma_start(out.ap()[:, :], outsb[:])

    nc.compile()
    return nc


def _shard_plan(cluster_id):
    cid = np.asarray(cluster_id).astype(np.int64).reshape(N_TOTAL)
    counts = np.bincount(cid, minlength=K_CL).astype(np.int64)
    nblk = max(1, -(-int(counts.max()) // BLK))        # ceil(max/BLK)
    return cid, counts, nblk


def _prep_inputs(x_path, cluster_id, W1, b1, Wf, bf, Wa, ba, Wb, bb, Wc, bc):
    """Host-side sharding / marshalling. Returns (in_maps, nblk)."""
    cid, counts, nblk = _shard_plan(cluster_id)
    npad = nblk * BLK
    x = np.asarray(x_path, dtype=np.float32).reshape(N_TOTAL, DIN)
    xb16 = x.astype(ml_dtypes.bfloat16)

    W1 = np.asarray(W1, np.float32); b1 = np.asarray(b1, np.float32)
    Wf = np.asarray(Wf, np.float32); bf = np.asarray(bf, np.float32)
    Wa = np.asarray(Wa, np.float32); ba = np.asarray(ba, np.float32)
    Wb = np.asarray(Wb, np.float32); bb = np.asarray(bb, np.float32)
    Wc = np.asarray(Wc, np.float32)

    def tiled_T(M):  # [256,256] -> [128, 512]; [p, j*256+f] = M.T[j*128+p, f]
        return np.ascontiguousarray(
            M.T.reshape(2, 128, DHID).transpose(1, 0, 2)).reshape(128, 512)

    def tiled_v(v):  # [256] -> [128, 2]; [p, j] = v[j*128+p]
        return np.ascontiguousarray(v.reshape(2, 128).T)

    blob_base = np.zeros((128, NBLOB), np.float32)
    blob_base[:, OFF_WFT:OFF_WFT + 512] = tiled_T(Wf)
    blob_base[:, OFF_WAT:OFF_WAT + 512] = tiled_T(Wa)
    blob_base[:, OFF_WBT:OFF_WBT + 512] = tiled_T(Wb * 0.5)
    wcr = np.broadcast_to((Wc.ravel() * 0.5).reshape(2, 128, 1),
                          (2, 128, 128)).transpose(1, 0, 2)
    blob_base[:, OFF_WCR:OFF_WCR + 256] = wcr.reshape(128, 256)
    blob_base[:, OFF_BFC:OFF_BFC + 2] = tiled_v(bf)
    blob_base[:, OFF_BAC:OFF_BAC + 2] = tiled_v(ba)
    blob_base[:, OFF_BBC:OFF_BBC + 2] = tiled_v(bb * 0.5)
    blob_base[:, OFF_B1C:OFF_B1C + 2] = tiled_v(b1)

    # W1.T tiled: [p, c*256+m] = W1[m, c*128+p]
    w1tt = np.ascontiguousarray(
        W1.T.reshape(KC, 128, DHID).transpose(1, 0, 2)
    ).reshape(128, KC * DHID).astype(ml_dtypes.bfloat16)

    relu_b1 = np.maximum(b1, 0.0).astype(np.float32)

    in_maps = []
    for k in range(N_CORES):
        rows = np.nonzero(cid == k)[0]
        nk = len(rows)
        shard = np.zeros((npad, DIN), dtype=ml_dtypes.bfloat16)
        shard[:nk] = xb16[rows]
        # [npad, 1024] -> [p, b, c, r] -> flat [128, nblk*8*512]
        xcore = np.ascontiguousarray(
            shard.reshape(nblk, BLK, KC, 128).transpose(3, 0, 2, 1)
        ).reshape(128, nblk * KC * BLK)

        invc = np.float32(1.0 / max(float(counts[k]), 1.0))
        n_pad = float(npad - nk)
        blob_k = blob_base.copy()
        blob_k[:, OFF_CORR:OFF_CORR + 2] = tiled_v(
            (-invc * n_pad) * relu_b1)
        blob_k[:, OFF_INVC] = invc
        in_maps.append({"xb": xcore, "w1t": w1tt, "blob": blob_k})
    return in_maps, nblk


def kernel(**inputs):
    _, _, nblk = _shard_plan(inputs["cluster_id"])
    key = ("nc", nblk)
    if key not in _CACHE:
        _CACHE[key] = _build_nc(nblk)
        _CACHE["nc"] = _CACHE[key]       # convenience handle for test.py
    nc = _CACHE[key]
    in_maps, _ = _prep_inputs(**inputs)
    res = bass_utils.run_bass_kernel_spmd(
        nc, in_maps, core_ids=list(range(N_CORES)))
    return _combine([res.results[k]["out"] for k in range(N_CORES)])


def _combine(outs):
    """Host-side gather: softmax over per-cluster logits + weighted sum."""
    logits = np.array([float(np.asarray(o)[0, 2]) for o in outs],
                      dtype=np.float64)
    h_path = np.stack([np.asarray(o)[:, 0:2].T.reshape(DHID) for o in outs])
    w = np.exp(logits - logits.max())
    w /= w.sum()
    H = (w[:, None] * h_path.astype(np.float64)).sum(axis=0)
    return np.ascontiguousarray(H.reshape(1, DHID)).astype(np.float32)


# revision 5
# speedup vs baseline: 1.9548x; 1.0293x over previous
"""
DeepAttMISL segment-reduce kernel for Trainium2 (Bass/Tile), 8 NeuronCores.

Math (see reference):
  h        = relu(x @ W1.T + b1)                    x:[N,1024] -> h:[N,256]
  seg      = segment_sum(h, cluster_id, 8)          -> [8,256]
  h_clust  = seg / max(counts,1)
  h_path   = relu(h_clust @ Wf.T + bf)
  A        = softmax((tanh(h_path@Wa.T+ba) * sigmoid(h_path@Wb.T+bb)) @ Wc.T)
  H        = A @ h_path                             -> [1,256]

Sharding: BY CLUSTER, not by rows.  Core k receives ALL rows of cluster k
(host sorts rows by cluster_id), zero-padded to a fixed NPAD rows.  Each
core therefore owns its cluster's full segment sum locally and NO cross-core
collective is needed (the ncfw AllReduce costs 25-35us per op in this
runtime, plus a ~56us wake, and dominated the previous version's critical
path).  Each core runs the tiny gated-attention head for its own cluster and
outputs (logit_k, h_path_k); the host does the final 8-way softmax +
weighted sum as the gather/unshard step.

Main matmul is computed TRANSPOSED (W1 stationary, x moving, h.T in PSUM
[hid_half, rows]) so the segment sum falls out of ACT's accum_out: one
activation op per PSUM tile does bias + relu + sum-over-rows.  No segment
matmuls, no one-hot matrix.  Zero-pad rows contribute exactly relu(b1)
each; the host bakes -n_pad*relu(b1)/count into a per-core correction.

x is streamed as NBLK contiguous 1MiB DMAs (8KiB per partition per block)
on the sync ring - near line rate.  bf16 everywhere in the big matmul
(fp8 fails the 2e-2 gate: W1's quantization error is shared across
instances so it does not average out); fp32 head.  sigmoid(y) =
0.5*(1+tanh(y/2)) with the 0.5 folded into Wc so one ACT table set
(relu/tanh/exp) serves the whole kernel.
"""

import sys

if "/opt/trn_rl_repo" not in sys.path:
    sys.path.insert(0, "/opt/trn_rl_repo")

import numpy as np
import ml_dtypes

import concourse.bass as bass
import concourse.tile as tile
from concourse import bacc, mybir
from concourse import bass_utils

ALU = mybir.AluOpType

N_CORES = 8
N_TOTAL = 65536
DIN = 1024
DHID = 256
K_CL = 8
KC = DIN // 128                        # 8 contraction chunks of 128
BLK = 512                              # rows per block (PSUM bank = 512 fp32)
WARMUP_MMS = 7                         # PE bridge: engine free ~8.4us, data
                                       # ~11.5us; 7 cold N=512 MMs = 3.0us

# blob layout (fp32 elements per partition)
OFF_WFT = 0                            # Wf.T tiled   [2,256] -> 512
OFF_WAT = 512                          # Wa.T tiled   [2,256] -> 512
OFF_WBT = 1024                         # (Wb/2).T     [2,256] -> 512
OFF_WCR = 1536                         # (Wc/2) bcast [2,128] -> 256
OFF_BFC = 1792                         # bf tiled     [2]
OFF_BAC = 1794                         # ba tiled     [2]
OFF_BBC = 1796                         # bb/2 tiled   [2]
OFF_B1C = 1798                         # b1 tiled     [2]
OFF_CORR = 1800                        # -invc*n_pad*relu(b1) tiled [2]
OFF_INVC = 1802                        # 1/max(count,1) scalar [1]
NBLOB = 1803

BF16 = mybir.dt.bfloat16
F32 = mybir.dt.float32
AF = mybir.ActivationFunctionType

_CACHE = {}


def _build_nc(nblk):
    npad = nblk * BLK
    nc = bacc.Bacc("TRN2", target_bir_lowering=False, debug=False,
                   num_devices=N_CORES)

    xb = nc.dram_tensor("xb", [128, nblk * KC * BLK], BF16,
                        kind="ExternalInput")
    w1t = nc.dram_tensor("w1t", [128, KC * DHID], BF16, kind="ExternalInput")
    blob = nc.dram_tensor("blob", [128, NBLOB], F32, kind="ExternalInput")
    out = nc.dram_tensor("out", [128, 4], F32, kind="ExternalOutput")

    with tile.TileContext(nc) as tc:
        with tc.tile_pool(name="consts", bufs=1) as consts, \
             tc.tile_pool(name="xblk", bufs=1) as xblk, \
             tc.tile_pool(name="hps", bufs=4, space="PSUM") as hps, \
             tc.tile_pool(name="headps", bufs=2, space="PSUM") as headps, \
             tc.tile_pool(name="small", bufs=1) as small:

            # ---- PE warm-up bridge: keep HAM busy (and un-throttled by the
            # time real data arrives) from t~0 until block 0 lands (~13us).
            wz = consts.tile([128, BLK], BF16)
            nc.vector.memset(wz[:], 0.0)
            wps = hps.tile([128, BLK], F32, tag="main")
            for _ in range(WARMUP_MMS):
                nc.tensor.matmul(wps[:], wz[:, 0:128], wz[:],
                                 start=True, stop=True, skip_group_check=True)

            # ---- DMAs: consts on the scalar ring (tiny bias slice first so
            # the first ACT isn't gated on the 1MiB head-weight blob, then
            # W1, then the head weights), all x blocks on the sync ring as
            # back-to-back 1MiB transfers (8KiB/partition lines).
            blob_sb = consts.tile([128, NBLOB], F32)
            nc.scalar.dma_start(blob_sb[:, OFF_BFC:], blob.ap()[:, OFF_BFC:])
            w1t_sb = consts.tile([128, KC, DHID], BF16)
            nc.scalar.dma_start(w1t_sb[:], w1t.ap().rearrange(
                "p (k f) -> p k f", k=KC))
            nc.scalar.dma_start(blob_sb[:, :OFF_BFC], blob.ap()[:, :OFF_BFC])

            xts = xblk.tile([128, nblk, KC, BLK], BF16)
            xv = xb.ap().rearrange("p (b k r) -> p b k r", b=nblk, k=KC)
            for b in range(nblk):
                nc.sync.dma_start(xts[:, b], xv[:, b])

            # ---- main loop: per block, per hid-half: 8 accumulated MMs
            # (W1 chunk stationary, x moving, N=512), then one ACT op doing
            # bias + relu + accum_out (the segment sum over this block).
            segparts = small.tile([128, 2, nblk], F32)
            hsc = [small.tile([128, BLK], BF16, name=f"hsc{i}")
                   for i in range(3)]
            for b in range(nblk):
                for j in range(2):
                    ps = hps.tile([128, BLK], F32, tag="main")
                    for c in range(KC):
                        nc.tensor.matmul(
                            ps[:],
                            w1t_sb[:, c, j * 128:(j + 1) * 128],
                            xts[:, b, c, :],
                            start=(c == 0), stop=(c == KC - 1),
                            skip_group_check=True)
                    nc.scalar.activation(
                        hsc[(2 * b + j) % 3][:], ps[:], AF.Relu,
                        bias=blob_sb[:, OFF_B1C + j:OFF_B1C + j + 1],
                        accum_out=segparts[:, j, b:b + 1])

            # ---- local segment sum -> cluster mean (with padding fix) ----
            seg2 = small.tile([128, 2], F32)
            for j in range(2):
                nc.vector.reduce_sum(seg2[:, j:j + 1], segparts[:, j, :],
                                     axis=mybir.AxisListType.X)
            hc = small.tile([128, 2], F32)
            nc.vector.tensor_scalar_mul(hc[:], seg2[:],
                                        blob_sb[:, OFF_INVC:OFF_INVC + 1])
            nc.vector.tensor_add(hc[:], hc[:],
                                 blob_sb[:, OFF_CORR:OFF_CORR + 2])

            # ---- gated-attention head for this core's cluster (fp32) ----
            def head_mm(w_off, rhs, b_off, func, name):
                o = small.tile([128, 2], F32, name=name)
                for j in range(2):
                    ps = headps.tile([128, 1], F32, tag="head",
                                     padded_shape=[128, BLK])
                    for i in range(2):
                        nc.tensor.matmul(
                            ps[:],
                            blob_sb[:, w_off + i * 256 + j * 128:
                                    w_off + i * 256 + (j + 1) * 128],
                            rhs[:, i:i + 1],
                            start=(i == 0), stop=(i == 1))
                    nc.scalar.activation(o[:, j:j + 1], ps[:], func,
                                         bias=blob_sb[:, b_off + j:
                                                      b_off + j + 1])
                return o

            hpT = head_mm(OFF_WFT, hc, OFF_BFC, AF.Relu, "hpT")
            aT = head_mm(OFF_WAT, hpT, OFF_BAC, AF.Tanh, "aT")
            tT = head_mm(OFF_WBT, hpT, OFF_BBC, AF.Tanh, "tT")
            # a*g = 0.5*a*(1+tanh(y/2)); the 0.5 lives in Wc/2
            ag = small.tile([128, 2], F32)
            nc.vector.tensor_mul(ag[:], aT[:], tT[:])
            nc.vector.tensor_add(ag[:], ag[:], aT[:])

            # logit (replicated across partitions via broadcast Wc/2)
            lps = headps.tile([128, 1], F32, tag="head",
                              padded_shape=[128, BLK])
            for j in range(2):
                nc.tensor.matmul(
                    lps[:],
                    blob_sb[:, OFF_WCR + j * 128:OFF_WCR + (j + 1) * 128],
                    ag[:, j:j + 1],
                    start=(j == 0), stop=(j == 1))

            # stream h_path out as soon as it's ready (overlaps the gate
            # matmuls); the logit follows in a second small DMA
            nc.sync.dma_start(out.ap()[:, 0:2], hpT[:])
            lsb = small.tile([128, 2], F32)
            nc.vector.tensor_copy(lsb[:, 0:1], lps[:])
            nc.vector.tensor_copy(lsb[:, 1:2], lps[:])
            nc.sync.dma_start(out.ap()[:, 2:4], lsb[:])

    nc.compile()
    return nc


def _shard_plan(cluster_id):
    cid = np.asarray(cluster_id).astype(np.int64).reshape(N_TOTAL)
    counts = np.bincount(cid, minlength=K_CL).astype(np.int64)
    nblk = max(1, -(-int(counts.max()) // BLK))        # ceil(max/BLK)
    return cid, counts, nblk


def _prep_inputs(x_path, cluster_id, W1, b1, Wf, bf, Wa, ba, Wb, bb, Wc, bc):
    """Host-side sharding / marshalling. Returns (in_maps, nblk)."""
    cid, counts, nblk = _shard_plan(cluster_id)
    npad = nblk * BLK
    x = np.asarray(x_path, dtype=np.float32).reshape(N_TOTAL, DIN)
    xb16 = x.astype(ml_dtypes.bfloat16)

    W1 = np.asarray(W1, np.float32); b1 = np.asarray(b1, np.float32)
    Wf = np.asarray(Wf, np.float32); bf = np.asarray(bf, np.float32)
    Wa = np.asarray(Wa, np.float32); ba = np.asarray(ba, np.float32)
    Wb = np.asarray(Wb, np.float32); bb = np.asarray(bb, np.float32)
    Wc = np.asarray(Wc, np.float32)

    def tiled_T(M):  # [256,256] -> [128, 512]; [p, j*256+f] = M.T[j*128+p, f]
        return np.ascontiguousarray(
            M.T.reshape(2, 128, DHID).transpose(1, 0, 2)).reshape(128, 512)

    def tiled_v(v):  # [256] -> [128, 2]; [p, j] = v[j*128+p]
        return np.ascontiguousarray(v.reshape(2, 128).T)

    blob_base = np.zeros((128, NBLOB), np.float32)
    blob_base[:, OFF_WFT:OFF_WFT + 512] = tiled_T(Wf)
    blob_base[:, OFF_WAT:OFF_WAT + 512] = tiled_T(Wa)
    blob_base[:, OFF_WBT:OFF_WBT + 512] = tiled_T(Wb * 0.5)
    wcr = np.broadcast_to((Wc.ravel() * 0.5).reshape(2, 128, 1),
                          (2, 128, 128)).transpose(1, 0, 2)
    blob_base[:, OFF_WCR:OFF_WCR + 256] = wcr.reshape(128, 256)
    blob_base[:, OFF_BFC:OFF_BFC + 2] = tiled_v(bf)
    blob_base[:, OFF_BAC:OFF_BAC + 2] = tiled_v(ba)
    blob_base[:, OFF_BBC:OFF_BBC + 2] = tiled_v(bb * 0.5)
    blob_base[:, OFF_B1C:OFF_B1C + 2] = tiled_v(b1)

    # W1.T tiled: [p, c*256+m] = W1[m, c*128+p]
    w1tt = np.ascontiguousarray(
        W1.T.reshape(KC, 128, DHID).transpose(1, 0, 2)
    ).reshape(128, KC * DHID).astype(ml_dtypes.bfloat16)

    relu_b1 = np.maximum(b1, 0.0).astype(np.float32)

    in_maps = []
    for k in range(N_CORES):
        rows = np.nonzero(cid == k)[0]
        nk = len(rows)
        shard = np.zeros((npad, DIN), dtype=ml_dtypes.bfloat16)
        shard[:nk] = xb16[rows]
        # [npad, 1024] -> [p, b, c, r] -> flat [128, nblk*8*512]
        xcore = np.ascontiguousarray(
            shard.reshape(nblk, BLK, KC, 128).transpose(3, 0, 2, 1)
        ).reshape(128, nblk * KC * BLK)

        invc = np.float32(1.0 / max(float(counts[k]), 1.0))
        n_pad = float(npad - nk)
        blob_k = blob_base.copy()
        blob_k[:, OFF_CORR:OFF_CORR + 2] = tiled_v(
            (-invc * n_pad) * relu_b1)
        blob_k[:, OFF_INVC] = invc
        in_maps.append({"xb": xcore, "w1t": w1tt, "blob": blob_k})
    return in_maps, nblk


def kernel(**inputs):
    _, _, nblk = _shard_plan(inputs["cluster_id"])
    key = ("nc", nblk)
    if key not in _CACHE:
        _CACHE[key] = _build_nc(nblk)
        _CACHE["nc"] = _CACHE[key]       # convenience handle for test.py
    nc = _CACHE[key]
    in_maps, _ = _prep_inputs(**inputs)
    res = bass_utils.run_bass_kernel_spmd(
        nc, in_maps, core_ids=list(range(N_CORES)))
    return _combine([res.results[k]["out"] for k in range(N_CORES)])


def _combine(outs):
    """Host-side gather: softmax over per-cluster logits + weighted sum."""
    logits = np.array([float(np.asarray(o)[0, 2]) for o in outs],
                      dtype=np.float64)
    h_path = np.stack([np.asarray(o)[:, 0:2].T.reshape(DHID) for o in outs])
    w = np.exp(logits - logits.max())
    w /= w.sum()
    H = (w[:, None] * h_path.astype(np.float64)).sum(axis=0)
    return np.ascontiguousarray(H.reshape(1, DHID)).astype(np.float32)
